# revision 8
# baseline (speedup 1.0000x reference)
"""GQA attention (dense_transformer) on 8 TRN2 NeuronCores, tensor-parallel.

Layout strategy (per core g of 8):
  - q-heads 4g..4g+3, kv-head g (GQA group == core).
  - Projections computed transposed: QT/KT [head_dim, S] via
    out = lhsT.T @ rhs with lhsT = weightT tiles, rhs = xT tiles.
  - Scores computed transposed: S^T[t, s] chunks [128, 512]; softmax sum via
    ones-matmul (partition reduction on PE); exp on ACT with fused 1/sqrt(dh)
    scale; causal handled by skipping above-diagonal chunks + 0/1 masks.
  - PV: out_T[dh, s] += V[t,dh].T-form matmuls; V obtained by PE-transposing VT.
  - attn heads all-gathered across cores (4 collectives, one per local head
    index) -> each core computes a 512-wide output-feature slice of the
    o-projection (wo col-shard). Host concatenates the 8 slices.
All matmuls run as float32r (1 cycle/row at N=512, ~4x faster than fp32).
"""

import numpy as np

import concourse.bass as bass
import concourse.mybir as mybir
import concourse.tile as tile
from concourse import bacc
from concourse.bass_utils import run_bass_kernel_spmd

F32 = mybir.dt.float32
F32R = mybir.dt.float32r

N_CORES = 8
S = 2048
DIM = 4096
DH = 128          # head dim
HQ_LOC = 4        # query heads per core
D_QKV = HQ_LOC * DH + 2 * DH   # 768 projection cols per core (4q + k + v)
SC = 512          # s-chunk
N_SC = S // SC    # 4
N_ET = DIM // 128  # 32 e-chunks
N_ST = S // 128    # 16 s-tiles
EOUT = 512        # output-feature slice per core
SCALE = 1.0 / np.sqrt(DH)

_NC_CACHE = None


def _r(ap):
    return ap if ap.dtype == F32R else ap.bitcast(F32R)


def build_nc():
    nc = bacc.Bacc("TRN2", target_bir_lowering=False, debug=False,
                   num_devices=N_CORES)

    xT = nc.declare_dram_parameter("xT", [DIM, S], F32R, isOutput=False)
    wqkvT = nc.declare_dram_parameter("wqkvT", [DIM, D_QKV], F32R, isOutput=False)
    woT = nc.declare_dram_parameter("woT", [DIM, EOUT], F32R, isOutput=False)
    cosT = nc.declare_dram_parameter("cosT", [DH, S], F32, isOutput=False)
    sinT = nc.declare_dram_parameter("sinT", [DH, S], F32, isOutput=False)
    rhT = nc.declare_dram_parameter("rhT", [DH, DH], F32R, isOutput=False)
    masks = nc.declare_dram_parameter("masks", [128, 4 * SC], F32, isOutput=False)
    ones = nc.declare_dram_parameter("ones", [128, 128], F32R, isOutput=False)
    ident = nc.declare_dram_parameter("ident", [128, 128], F32, isOutput=False)
    out_ext = nc.declare_dram_parameter("out", [S, EOUT], F32, isOutput=True)

    with tile.TileContext(nc) as tc, \
            tc.tile_pool(name="ag_dram", bufs=1, space="DRAM") as ag_dram:
        # phases 1+2 in their own scope so their SBUF frees before the o-proj
        with tc.tile_pool(name="const12", bufs=1) as cpool:
            cos_sb = cpool.tile([DH, S], F32, name="cos_sb")
            sin_sb = cpool.tile([DH, S], F32, name="sin_sb")
            rh_sb = cpool.tile([DH, DH], F32R, name="rh_sb")
            mask_sb = cpool.tile([128, 4 * SC], F32, name="mask_sb")
            ones_sb = cpool.tile([128, 128], F32R, name="ones_sb")
            id_sb = cpool.tile([128, 128], F32, name="id_sb")
            nc.sync.dma_start(cos_sb[:], cosT[:])
            nc.sync.dma_start(sin_sb[:], sinT[:])
            nc.sync.dma_start(rh_sb[:], rhT[:])
            nc.sync.dma_start(mask_sb[:], masks[:])
            nc.sync.dma_start(ones_sb[:], ones[:])
            nc.sync.dma_start(id_sb[:], ident[:])

            with tc.tile_pool(name="persist", bufs=1) as ppool:
                # rotated QT (4 heads) + KT : 5 tiles [128, 2048]
                qk_rot = [ppool.tile([DH, S], F32R, name=f"qkrot{d}")
                          for d in range(HQ_LOC + 1)]
                v_all = ppool.tile([128, N_ST * DH], F32R, name="v_all")  # [t,dh] blocks

                _phase_qkv(nc, tc, xT, wqkvT, cos_sb, sin_sb, rh_sb, id_sb,
                           qk_rot, v_all)

                ag_outs = _phase_attention(nc, tc, qk_rot, v_all, ones_sb,
                                           mask_sb, ag_dram)

        _phase_oproj(nc, tc, ag_outs, woT, out_ext)

    nc.compile()
    return nc


def _phase_qkv(nc, tc, xT, wqkvT, cos_sb, sin_sb, rh_sb, id_sb, qk_rot, v_all):
    """QT/KT/VT projections + RoPE + V transpose."""
    with (
        tc.tile_pool(name="wpool", bufs=1) as wpool,
        tc.tile_pool(name="xpool", bufs=3) as xpool,
        tc.tile_pool(name="evpool", bufs=3) as evpool,
        tc.tile_pool(name="tmppool", bufs=3) as tmppool,
        tc.tile_pool(name="qkv_psum", bufs=1, space="PSUM") as qk_pp,
        tc.tile_pool(name="rh_psum", bufs=2, space="PSUM") as rh_pp,
    ):
        w_all = wpool.tile([128, N_ET * D_QKV], F32R, name="w_all")
        for e in range(N_ET):
            nc.sync.dma_start(w_all[:, e * D_QKV:(e + 1) * D_QKV],
                              wqkvT[e * 128:(e + 1) * 128, :])

        for c in range(N_SC):
            ssl = slice(c * SC, (c + 1) * SC)
            accs = [qk_pp.tile([128, SC], F32, tag=f"acc{d}", name=f"acc{d}_{c}")
                    for d in range(6)]
            for e in range(N_ET):
                xt = xpool.tile([128, SC], F32R, tag="xt", name=f"xt{c}_{e}")
                nc.sync.dma_start(xt[:], xT[e * 128:(e + 1) * 128, ssl])
                for d in range(6):
                    lhsT = w_all[:, e * D_QKV + d * 128: e * D_QKV + (d + 1) * 128]
                    nc.tensor.matmul(accs[d][:], _r(lhsT), _r(xt[:]),
                                     start=(e == 0), stop=(e == N_ET - 1))
            # RoPE for Q0..Q3, K
            for d in range(5):
                ev = evpool.tile([128, SC], F32R, tag="ev", name=f"ev{c}_{d}")
                nc.vector.tensor_copy(ev[:], accs[d][:])
                rhp = rh_pp.tile([128, SC], F32, tag="rh", name=f"rhp{c}_{d}")
                nc.tensor.matmul(rhp[:], _r(rh_sb[:]), _r(ev[:]),
                                 start=True, stop=True)
                tmp = tmppool.tile([128, SC], F32, tag="tmp", name=f"tmp{c}_{d}")
                nc.vector.tensor_mul(tmp[:], rhp[:], sin_sb[:, ssl])
                dst = qk_rot[d][:, ssl]
                nc.vector.tensor_mul(dst, ev[:], cos_sb[:, ssl])
                nc.vector.tensor_add(dst, dst, tmp[:])
            # V: evict, then transpose [dh, t]->[t, dh] per 128-block
            vt = evpool.tile([128, SC], F32, tag="vt", name=f"vt{c}")
            nc.vector.tensor_copy(vt[:], accs[5][:])
            for tb in range(SC // 128):
                t_ix = c * (SC // 128) + tb
                trp = rh_pp.tile([128, 128], F32, tag="rh", name=f"tr{t_ix}")
                nc.tensor.transpose(trp[:], vt[:, tb * 128:(tb + 1) * 128],
                                    id_sb[:])
                nc.vector.tensor_copy(v_all[:, t_ix * DH:(t_ix + 1) * DH],
                                      trp[:])


def _phase_attention(nc, tc, qk_rot, v_all, ones_sb, mask_sb, agpool):
    """Causal attention per local head; returns list of 4 gathered DRAM tiles."""
    ag_outs = []
    with (
        tc.tile_pool(name="ptpool", bufs=2) as ptpool,
        tc.tile_pool(name="attnpool", bufs=3) as apool,
        tc.tile_pool(name="recippool", bufs=2) as rpool,
        tc.tile_pool(name="s_psum", bufs=2, space="PSUM") as s_pp,
        tc.tile_pool(name="l_psum", bufs=1, space="PSUM") as l_pp,
        tc.tile_pool(name="o_psum", bufs=2, space="PSUM") as o_pp,
    ):
        kt = qk_rot[HQ_LOC]
        for h in range(HQ_LOC):
            ag_in = agpool.tile([DH, S], F32, name=f"ag_in{h}")
            ag_out = agpool.tile([N_CORES * DH, S], F32, addr_space="Shared",
                                 name=f"ag_out{h}")
            qt = qk_rot[h]
            for c in range(N_SC):
                ssl = slice(c * SC, (c + 1) * SC)
                n_tb = 4 * c + 4
                pt = ptpool.tile([128, n_tb * SC], F32R, tag="pt",
                                 name=f"pt{h}_{c}")
                ps_l = l_pp.tile([128, SC], F32, tag="l", name=f"l{h}_{c}")
                ps_o = o_pp.tile([128, SC], F32, tag="o", name=f"o{h}_{c}")
                for m in range(n_tb // 2):
                    ps = s_pp.tile([128, 2 * SC], F32, tag="ps",
                                   name=f"ps{h}_{c}_{m}")
                    for q in range(2):
                        tb = 2 * m + q
                        nc.tensor.matmul(
                            ps[:, q * SC:(q + 1) * SC],
                            _r(kt[:, tb * 128:(tb + 1) * 128]),
                            _r(qt[:, ssl]), start=True, stop=True)
                    nc.scalar.activation(pt[:, 2 * m * SC:(2 * m + 2) * SC],
                                         ps[:], mybir.ActivationFunctionType.Exp,
                                         scale=float(SCALE))
                    for q in range(2):
                        tb = 2 * m + q
                        p = tb - 4 * c
                        if p >= 0:  # diagonal-region chunk: 0/1 causal mask
                            sl = slice(tb * SC, (tb + 1) * SC)
                            nc.vector.tensor_mul(
                                pt[:, sl], pt[:, sl],
                                mask_sb[:, p * SC:(p + 1) * SC])
                for tb in range(n_tb):
                    sl = slice(tb * SC, (tb + 1) * SC)
                    nc.tensor.matmul(ps_l[:], _r(ones_sb[:]), _r(pt[:, sl]),
                                     start=(tb == 0), stop=(tb == n_tb - 1))
                    nc.tensor.matmul(ps_o[:],
                                     _r(v_all[:, tb * DH:(tb + 1) * DH]),
                                     _r(pt[:, sl]),
                                     start=(tb == 0), stop=(tb == n_tb - 1))
                recip = rpool.tile([128, SC], F32, tag="recip",
                                   name=f"recip{h}_{c}")
                nc.vector.reciprocal(recip[:], ps_l[:])
                attn = apool.tile([128, SC], F32, tag="attn",
                                  name=f"attn{h}_{c}")
                nc.vector.tensor_mul(attn[:], ps_o[:], recip[:])
                nc.gpsimd.dma_start(ag_in[:, ssl], attn[:])
            nc.gpsimd.collective_compute(
                "AllGather", mybir.AluOpType.bypass,
                replica_groups=[list(range(N_CORES))],
                ins=[ag_in.opt()], outs=[ag_out.opt()])
            ag_outs.append(ag_out)
    return ag_outs


def _phase_oproj(nc, tc, ag_outs, woT, out_ext):
    """out[:, 512g:512g+512] = attnT_full.T @ woT_g, accumulated over 4 AG groups."""
    with (
        tc.tile_pool(name="wopool", bufs=1) as wopool,
        tc.tile_pool(name="agpool_sb", bufs=1) as agsb,
        tc.tile_pool(name="oaccpool", bufs=1) as oacc,
        tc.tile_pool(name="po_psum", bufs=2, space="PSUM") as po_pp,
    ):
        wo_sb = {}
        for k in range(HQ_LOC):
            for j in range(N_CORES):
                t = wopool.tile([128, EOUT], F32R, name=f"wo{k}_{j}")
                nc.sync.dma_start(
                    t[:], woT[512 * j + 128 * k: 512 * j + 128 * (k + 1), :])
                wo_sb[(k, j)] = t
        out_acc = [oacc.tile([128, EOUT], F32, name=f"oacc{st}")
                   for st in range(N_ST)]
        for k in range(HQ_LOC):
            ag_sb = []
            for j in range(N_CORES):
                t = agsb.tile([128, S], F32R, tag=f"ag{j}", name=f"agsb{k}_{j}")
                nc.gpsimd.dma_start(t[:], ag_outs[k][128 * j:128 * (j + 1), :].bitcast(F32R))
                ag_sb.append(t)
            for st in range(N_ST):
                po = po_pp.tile([128, EOUT], F32, tag="po", name=f"po{k}_{st}")
                for j in range(N_CORES):
                    nc.tensor.matmul(
                        po[:], _r(ag_sb[j][:, st * 128:(st + 1) * 128]),
                        _r(wo_sb[(k, j)][:]),
                        start=(j == 0), stop=(j == N_CORES - 1))
                if k == 0:
                    nc.vector.tensor_copy(out_acc[st][:], po[:])
                else:
                    nc.vector.tensor_add(out_acc[st][:], out_acc[st][:], po[:])
        for st in range(N_ST):
            nc.sync.dma_start(out_ext[st * 128:(st + 1) * 128, :],
                              out_acc[st][:])


def make_host_inputs(x, wq, wk, wv, wo, rope_freqs):
    """Shard + pre-transpose inputs; returns list of 8 in_maps."""
    x2 = np.asarray(x, np.float32).reshape(S, DIM)
    xT = np.ascontiguousarray(x2.T)
    rope = np.asarray(rope_freqs, np.float32)
    cosT = np.ascontiguousarray(rope[:, :, 0].T)
    sinT = np.ascontiguousarray(rope[:, :, 1].T)

    rh = np.zeros((DH, DH), np.float32)
    ii = np.arange(0, DH, 2)
    rh[ii, ii + 1] = -1.0   # out[2i] = -in[2i+1]
    rh[ii + 1, ii] = 1.0    # out[2i+1] = in[2i]
    rhT = np.ascontiguousarray(rh.T)

    t_ix = np.arange(128)[:, None]
    s_ix = np.arange(SC)[None, :]
    masks = np.empty((128, 4 * SC), np.float32)
    for p in range(4):
        # chunk tb at diag position p: s-blocks < p invalid; block p triangular
        valid = (s_ix - p * 128) >= t_ix
        masks[:, p * SC:(p + 1) * SC] = valid.astype(np.float32)
    ones = np.ones((128, 128), np.float32)
    ident = np.eye(128, dtype=np.float32)

    woT = np.ascontiguousarray(np.asarray(wo, np.float32).T)
    in_maps = []
    for g in range(N_CORES):
        wq_g = wq[512 * g:512 * (g + 1)]
        wk_g = wk[128 * g:128 * (g + 1)]
        wv_g = wv[128 * g:128 * (g + 1)]
        wqkvT = np.ascontiguousarray(
            np.concatenate([wq_g, wk_g, wv_g], axis=0).astype(np.float32).T)
        woT_g = np.ascontiguousarray(woT[:, EOUT * g:EOUT * (g + 1)])
        in_maps.append({
            "xT": xT, "wqkvT": wqkvT, "woT": woT_g, "cosT": cosT,
            "sinT": sinT, "rhT": rhT, "masks": masks, "ones": ones,
            "ident": ident,
        })
    return in_maps


def get_nc():
    global _NC_CACHE
    if _NC_CACHE is None:
        _NC_CACHE = build_nc()
    return _NC_CACHE


def kernel(x, wq, wk, wv, wo, rope_freqs, start_pos=0, **_unused):
    nc = get_nc()
    in_maps = make_host_inputs(x, wq, wk, wv, wo, rope_freqs)
    res = run_bass_kernel_spmd(nc, in_maps, core_ids=list(range(N_CORES)))
    out = np.concatenate([res.results[g]["out"] for g in range(N_CORES)],
                         axis=1)
    return out.reshape(1, S, DIM).astype(np.float32)


# revision 13
# speedup vs baseline: 2.8169x; 2.8169x over previous
"""GQA attention (dense_transformer) on 8 TRN2 NeuronCores, tensor-parallel.

Layout strategy (per core g of 8):
  - q-heads 4g..4g+3, kv-head g (GQA group == core).
  - Projections computed transposed: QT/KT [head_dim, S] via
    out = lhsT.T @ rhs with lhsT = weightT tiles, rhs = xT tiles.
  - Scores computed transposed: S^T[t, s] chunks [128, 512]; softmax sum via
    ones-matmul (partition reduction on PE); exp on ACT with fused 1/sqrt(dh)
    scale; causal handled by skipping above-diagonal chunks + 0/1 masks.
  - PV: out_T[dh, s] += V[t,dh].T-form matmuls; V obtained by PE-transposing VT.
  - attn heads all-gathered across cores (4 collectives, one per local head
    index) -> each core computes a 512-wide output-feature slice of the
    o-projection (wo col-shard). Host concatenates the 8 slices.
All matmuls run as float32r (1 cycle/row at N=512, ~4x faster than fp32).
"""

import numpy as np

import concourse.bass as bass
import concourse.mybir as mybir
import concourse.tile as tile
from concourse import bacc
from concourse.bass_utils import run_bass_kernel_spmd

F32 = mybir.dt.float32
F32R = mybir.dt.float32r

N_CORES = 8
S = 2048
DIM = 4096
DH = 128          # head dim
HQ_LOC = 4        # query heads per core
D_QKV = HQ_LOC * DH + 2 * DH   # 768 projection cols per core (4q + k + v)
SC = 512          # s-chunk
N_SC = S // SC    # 4
N_ET = DIM // 128  # 32 e-chunks
N_ST = S // 128    # 16 s-tiles
EOUT = 512        # output-feature slice per core
SCALE = 1.0 / np.sqrt(DH)

_NC_CACHE = None


def _r(ap):
    return ap if ap.dtype == F32R else ap.bitcast(F32R)


def build_nc(repeat=1, n_cores=N_CORES, phases=(1, 2, 3)):
    nc = bacc.Bacc("TRN2", target_bir_lowering=False, debug=False,
                   num_devices=n_cores)

    xT = nc.declare_dram_parameter("xT", [DIM, S], F32R, isOutput=False)
    wqkvT = nc.declare_dram_parameter("wqkvT", [DIM, D_QKV], F32R, isOutput=False)
    woT = nc.declare_dram_parameter("woT", [DIM, EOUT], F32R, isOutput=False)
    cosT = nc.declare_dram_parameter("cosT", [DH, S], F32, isOutput=False)
    sinT = nc.declare_dram_parameter("sinT", [DH, S], F32, isOutput=False)
    rhT = nc.declare_dram_parameter("rhT", [DH, DH], F32R, isOutput=False)
    masks = nc.declare_dram_parameter("masks", [128, 4 * SC], F32, isOutput=False)
    ones = nc.declare_dram_parameter("ones", [128, 128], F32R, isOutput=False)
    ident = nc.declare_dram_parameter("ident", [128, 128], F32, isOutput=False)
    out_ext = nc.declare_dram_parameter("out", [S, EOUT], F32, isOutput=True)

    with tile.TileContext(nc) as tc, \
            tc.tile_pool(name="ag_dram", bufs=1, space="DRAM") as ag_dram:
        for rep in range(repeat):
            p = f"r{rep}_"
            # phases 1+2 scoped so their SBUF frees before the o-proj
            with tc.tile_pool(name=p + "const12", bufs=1) as cpool:
                cos_sb = cpool.tile([DH, S], F32, name=p + "cos_sb")
                sin_sb = cpool.tile([DH, S], F32, name=p + "sin_sb")
                rh_sb = cpool.tile([DH, DH], F32R, name=p + "rh_sb")
                mask_sb = cpool.tile([128, 4 * SC], F32, name=p + "mask_sb")
                ones_sb = cpool.tile([128, 128], F32R, name=p + "ones_sb")
                id_sb = cpool.tile([128, 128], F32, name=p + "id_sb")
                nc.sync.dma_start(cos_sb[:], cosT[:])
                nc.sync.dma_start(sin_sb[:], sinT[:])
                nc.sync.dma_start(rh_sb[:], rhT[:])
                nc.sync.dma_start(mask_sb[:], masks[:])
                nc.sync.dma_start(ones_sb[:], ones[:])
                nc.sync.dma_start(id_sb[:], ident[:])

                with tc.tile_pool(name=p + "persist", bufs=1) as ppool:
                    # rotated QT (4 heads) + KT : 5 tiles [128, 2048]
                    qk_rot = [ppool.tile([DH, S], F32R, name=f"{p}qkrot{d}")
                              for d in range(HQ_LOC + 1)]
                    v_all = ppool.tile([128, N_ST * DH], F32R,
                                       name=p + "v_all")  # [t,dh] blocks

                    if 1 in phases:
                        _phase_qkv(nc, tc, xT, wqkvT, cos_sb, sin_sb, rh_sb,
                                   id_sb, qk_rot, v_all, p)

                    ag_outs = None
                    if 2 in phases:
                        ag_outs = _phase_attention(
                            nc, tc, qk_rot, v_all, ones_sb, mask_sb, ag_dram,
                            p, n_cores)

            if 3 in phases and ag_outs is not None:
                _phase_oproj(nc, tc, ag_outs, woT, out_ext, p, n_cores)

    nc.compile()
    return nc


def _phase_qkv(nc, tc, xT, wqkvT, cos_sb, sin_sb, rh_sb, id_sb, qk_rot, v_all,
               pfx=""):
    """QT/KT/VT projections + RoPE + V transpose."""
    with (
        tc.tile_pool(name=pfx + "wpool", bufs=1) as wpool,
        tc.tile_pool(name=pfx + "xpool", bufs=3) as xpool,
        tc.tile_pool(name=pfx + "evpool", bufs=3) as evpool,
        tc.tile_pool(name=pfx + "tmppool", bufs=3) as tmppool,
        tc.tile_pool(name=pfx + "qkv_psum", bufs=1, space="PSUM") as qk_pp,
        tc.tile_pool(name=pfx + "rh_psum", bufs=2, space="PSUM") as rh_pp,
    ):
        w_all = wpool.tile([128, N_ET * D_QKV], F32R, name=pfx + "w_all")
        for e in range(N_ET):
            nc.sync.dma_start(w_all[:, e * D_QKV:(e + 1) * D_QKV],
                              wqkvT[e * 128:(e + 1) * 128, :])

        for c in range(N_SC):
            ssl = slice(c * SC, (c + 1) * SC)
            accs = [qk_pp.tile([128, SC], F32, tag=f"acc{d}", name=f"{pfx}acc{d}_{c}")
                    for d in range(6)]
            for e in range(N_ET):
                xt = xpool.tile([128, SC], F32R, tag="xt", name=f"{pfx}xt{c}_{e}")
                nc.sync.dma_start(xt[:], xT[e * 128:(e + 1) * 128, ssl])
                for d in range(6):
                    lhsT = w_all[:, e * D_QKV + d * 128: e * D_QKV + (d + 1) * 128]
                    nc.tensor.matmul(accs[d][:], _r(lhsT), _r(xt[:]),
                                     start=(e == 0), stop=(e == N_ET - 1))
            # RoPE for Q0..Q3, K
            for d in range(5):
                ev = evpool.tile([128, SC], F32R, tag="ev", name=f"{pfx}ev{c}_{d}")
                nc.vector.tensor_copy(ev[:], accs[d][:])
                rhp = rh_pp.tile([128, SC], F32, tag="rh", name=f"{pfx}rhp{c}_{d}")
                nc.tensor.matmul(rhp[:], _r(rh_sb[:]), _r(ev[:]),
                                 start=True, stop=True)
                tmp = tmppool.tile([128, SC], F32, tag="tmp", name=f"{pfx}tmp{c}_{d}")
                nc.vector.tensor_mul(tmp[:], rhp[:], sin_sb[:, ssl])
                dst = qk_rot[d][:, ssl]
                nc.vector.tensor_mul(dst, ev[:], cos_sb[:, ssl])
                nc.vector.tensor_add(dst, dst, tmp[:])
            # V: evict, then transpose [dh, t]->[t, dh] per 128-block
            vt = evpool.tile([128, SC], F32, tag="vt", name=f"{pfx}vt{c}")
            nc.vector.tensor_copy(vt[:], accs[5][:])
            for tb in range(SC // 128):
                t_ix = c * (SC // 128) + tb
                trp = rh_pp.tile([128, 128], F32, tag="rh", name=f"{pfx}tr{t_ix}")
                nc.tensor.transpose(trp[:], vt[:, tb * 128:(tb + 1) * 128],
                                    id_sb[:])
                nc.vector.tensor_copy(v_all[:, t_ix * DH:(t_ix + 1) * DH],
                                      trp[:])


def _phase_attention(nc, tc, qk_rot, v_all, ones_sb, mask_sb, agpool, pfx="",
                     n_cores=N_CORES):
    """Causal attention per local head; returns list of 4 gathered DRAM tiles."""
    ag_outs = []
    with (
        tc.tile_pool(name=pfx + "ptpool", bufs=2) as ptpool,
        tc.tile_pool(name=pfx + "attnpool", bufs=3) as apool,
        tc.tile_pool(name=pfx + "recippool", bufs=2) as rpool,
        tc.tile_pool(name=pfx + "s_psum", bufs=2, space="PSUM") as s_pp,
        tc.tile_pool(name=pfx + "l_psum", bufs=1, space="PSUM") as l_pp,
        tc.tile_pool(name=pfx + "o_psum", bufs=2, space="PSUM") as o_pp,
    ):
        kt = qk_rot[HQ_LOC]
        for h in range(HQ_LOC):
            ag_in = agpool.tile([DH, S], F32, name=f"{pfx}ag_in{h}")
            ag_out = agpool.tile([N_CORES * DH, S], F32,
                                 addr_space="Shared" if n_cores > 1 else "Local",
                                 name=f"{pfx}ag_out{h}")
            qt = qk_rot[h]
            for c in range(N_SC):
                ssl = slice(c * SC, (c + 1) * SC)
                n_tb = 4 * c + 4
                pt = ptpool.tile([128, n_tb * SC], F32R, tag="pt",
                                 name=f"{pfx}pt{h}_{c}")
                ps_l = l_pp.tile([128, SC], F32, tag="l", name=f"{pfx}l{h}_{c}")
                ps_o = o_pp.tile([128, SC], F32, tag="o", name=f"{pfx}o{h}_{c}")
                for m in range(n_tb // 2):
                    ps = s_pp.tile([128, 2 * SC], F32, tag="ps",
                                   name=f"{pfx}ps{h}_{c}_{m}")
                    for q in range(2):
                        tb = 2 * m + q
                        nc.tensor.matmul(
                            ps[:, q * SC:(q + 1) * SC],
                            _r(kt[:, tb * 128:(tb + 1) * 128]),
                            _r(qt[:, ssl]), start=True, stop=True)
                    nc.scalar.activation(pt[:, 2 * m * SC:(2 * m + 2) * SC],
                                         ps[:], mybir.ActivationFunctionType.Exp,
                                         scale=float(SCALE))
                    for q in range(2):
                        tb = 2 * m + q
                        dpos = tb - 4 * c
                        if dpos >= 0:  # diagonal-region chunk: 0/1 causal mask
                            sl = slice(tb * SC, (tb + 1) * SC)
                            nc.vector.tensor_mul(
                                pt[:, sl], pt[:, sl],
                                mask_sb[:, dpos * SC:(dpos + 1) * SC])
                for tb in range(n_tb):
                    sl = slice(tb * SC, (tb + 1) * SC)
                    nc.tensor.matmul(ps_l[:], _r(ones_sb[:]), _r(pt[:, sl]),
                                     start=(tb == 0), stop=(tb == n_tb - 1))
                    nc.tensor.matmul(ps_o[:],
                                     _r(v_all[:, tb * DH:(tb + 1) * DH]),
                                     _r(pt[:, sl]),
                                     start=(tb == 0), stop=(tb == n_tb - 1))
                recip = rpool.tile([128, SC], F32, tag="recip",
                                   name=f"{pfx}recip{h}_{c}")
                nc.vector.reciprocal(recip[:], ps_l[:])
                attn = apool.tile([128, SC], F32, tag="attn",
                                  name=f"{pfx}attn{h}_{c}")
                nc.vector.tensor_mul(attn[:], ps_o[:], recip[:])
                nc.gpsimd.dma_start(ag_in[:, ssl], attn[:])
            if n_cores > 1:
                nc.gpsimd.collective_compute(
                    "AllGather", mybir.AluOpType.bypass,
                    replica_groups=[list(range(n_cores))],
                    ins=[ag_in.opt()], outs=[ag_out.opt()])
            else:
                nc.gpsimd.dma_start(ag_out[0:DH, :], ag_in[:])
            ag_outs.append(ag_out)
    return ag_outs


def _phase_oproj(nc, tc, ag_outs, woT, out_ext, pfx="", n_cores=N_CORES):
    """out[:, 512g:512g+512] = attnT_full.T @ woT_g, accumulated over 4 AG groups."""
    with (
        tc.tile_pool(name=pfx + "wopool", bufs=1) as wopool,
        tc.tile_pool(name=pfx + "agpool_sb", bufs=1) as agsb,
        tc.tile_pool(name=pfx + "oaccpool", bufs=1) as oacc,
        tc.tile_pool(name=pfx + "po_psum", bufs=2, space="PSUM") as po_pp,
    ):
        wo_sb = {}
        for k in range(HQ_LOC):
            for j in range(N_CORES):
                t = wopool.tile([128, EOUT], F32R, name=f"{pfx}wo{k}_{j}")
                nc.sync.dma_start(
                    t[:], woT[512 * j + 128 * k: 512 * j + 128 * (k + 1), :])
                wo_sb[(k, j)] = t
        out_acc = [oacc.tile([128, EOUT], F32, name=f"{pfx}oacc{st}")
                   for st in range(N_ST)]
        for k in range(HQ_LOC):
            ag_sb = []
            for j in range(N_CORES):
                t = agsb.tile([128, S], F32R, tag=f"ag{j}", name=f"{pfx}agsb{k}_{j}")
                nc.gpsimd.dma_start(t[:], ag_outs[k][128 * j:128 * (j + 1), :].bitcast(F32R))
                ag_sb.append(t)
            for st in range(N_ST):
                po = po_pp.tile([128, EOUT], F32, tag="po", name=f"{pfx}po{k}_{st}")
                for j in range(N_CORES):
                    nc.tensor.matmul(
                        po[:], _r(ag_sb[j][:, st * 128:(st + 1) * 128]),
                        _r(wo_sb[(k, j)][:]),
                        start=(j == 0), stop=(j == N_CORES - 1))
                if k == 0:
                    nc.vector.tensor_copy(out_acc[st][:], po[:])
                else:
                    nc.vector.tensor_add(out_acc[st][:], out_acc[st][:], po[:])
        for st in range(N_ST):
            nc.sync.dma_start(out_ext[st * 128:(st + 1) * 128, :],
                              out_acc[st][:])


def make_host_inputs(x, wq, wk, wv, wo, rope_freqs):
    """Shard + pre-transpose inputs; returns list of 8 in_maps."""
    x2 = np.asarray(x, np.float32).reshape(S, DIM)
    xT = np.ascontiguousarray(x2.T)
    rope = np.asarray(rope_freqs, np.float32)
    cosT = np.ascontiguousarray(rope[:, :, 0].T)
    sinT = np.ascontiguousarray(rope[:, :, 1].T)

    rh = np.zeros((DH, DH), np.float32)
    ii = np.arange(0, DH, 2)
    rh[ii, ii + 1] = -1.0   # out[2i] = -in[2i+1]
    rh[ii + 1, ii] = 1.0    # out[2i+1] = in[2i]
    rhT = np.ascontiguousarray(rh.T)

    t_ix = np.arange(128)[:, None]
    s_ix = np.arange(SC)[None, :]
    masks = np.empty((128, 4 * SC), np.float32)
    for p in range(4):
        # chunk tb at diag position p: s-blocks < p invalid; block p triangular
        valid = (s_ix - p * 128) >= t_ix
        masks[:, p * SC:(p + 1) * SC] = valid.astype(np.float32)
    ones = np.ones((128, 128), np.float32)
    ident = np.eye(128, dtype=np.float32)

    woT = np.ascontiguousarray(np.asarray(wo, np.float32).T)
    in_maps = []
    for g in range(N_CORES):
        wq_g = wq[512 * g:512 * (g + 1)]
        wk_g = wk[128 * g:128 * (g + 1)]
        wv_g = wv[128 * g:128 * (g + 1)]
        wqkvT = np.ascontiguousarray(
            np.concatenate([wq_g, wk_g, wv_g], axis=0).astype(np.float32).T)
        woT_g = np.ascontiguousarray(woT[:, EOUT * g:EOUT * (g + 1)])
        in_maps.append({
            "xT": xT, "wqkvT": wqkvT, "woT": woT_g, "cosT": cosT,
            "sinT": sinT, "rhT": rhT, "masks": masks, "ones": ones,
            "ident": ident,
        })
    return in_maps


def get_nc():
    global _NC_CACHE
    if _NC_CACHE is None:
        _NC_CACHE = build_nc()
    return _NC_CACHE


def kernel(x, wq, wk, wv, wo, rope_freqs, start_pos=0, **_unused):
    nc = get_nc()
    in_maps = make_host_inputs(x, wq, wk, wv, wo, rope_freqs)
    res = run_bass_kernel_spmd(nc, in_maps, core_ids=list(range(N_CORES)))
    out = np.concatenate([res.results[g]["out"] for g in range(N_CORES)],
                         axis=1)
    return out.reshape(1, S, DIM).astype(np.float32)
